# revision 35
# baseline (speedup 1.0000x reference)
"""Committee-vote histogram kernel for TRN2 (8 NeuronCores, data-parallel).

votes[b, c] = sum_m 1[argmax_c' (x[b] @ W[m, :, c'] + b[m, c']) == c]

Strategy per core (batch shard of 8192 rows):
  - x is decomposed host-side into an exact fp16 pair (x = xh + xl with
    residual ~2^-22|x|); likewise W and the bias. Logits are computed as
    xh@Wh + xh@Wl + xl@Wh (+bias), whose decomposition error (~2e-7) is at
    fp32 rounding level — validated exact-match against the fp32 reference.
  - All input DMAs are issued up front across both HWDGE queues (sync: xh
    stream, scalar: xl stream); the first chunk is split by contraction
    half (k) so the PE can start after only 0.5 MB has landed.
  - While the first chunk is in flight the PE runs warm-up matmuls on a
    scratch PSUM tile so the p-state clock ramp starts before real work.
  - The two xh passes are FUSED into one matmul per (tile, k): rhs is the
    concatenated (wh|wl) 160 columns and the out AP carries a stride-0
    dim that folds columns 80..159 back onto 0..79, so both products
    accumulate into the same PSUM cells (start=False => every column
    write accumulates). This cuts matmul + LDWEIGHTS count by a third.
  - Logits accumulate in PSUM as [128 b, 2 banks x 4 tiles x 80 (m,c)]
    per 8-tile super-batch; one DVE chain per super-batch (5D APs):
    reduce_max, fp16 is_ge mask (unit-stride), then a 3-step unit-stride
    fp16 add tree for the member sum — into a single fp16 staging tile,
    stored with one DMA at the end. The host de-interleaves
    [p, tile, c] -> [b, c] and casts to f32 (exact for counts <= 8).
"""

import os
import sys

import numpy as np

if os.path.isdir("/opt/trn_rl_repo") and "/opt/trn_rl_repo" not in sys.path:
    sys.path.insert(0, "/opt/trn_rl_repo")

import concourse.bass as bass
import concourse.tile as tile
from concourse import bacc, mybir
from concourse.bass import ts

F32 = mybir.dt.float32
F16 = mybir.dt.float16

B_FULL = 65536
D = 256
C = 10
M = 8
N_CORES = 8
B_SHARD = B_FULL // N_CORES  # 8192
P = 128

MC = M * C  # 80 logit columns per sample
CHUNKS = (1024, 2048, 2048, 2048, 1024)  # rows per input-DMA chunk
WARMUP_MMS = 8


def build_nc(b_shard: int = B_SHARD) -> bass.Bass:
    assert sum(CHUNKS) == b_shard
    n_tiles = b_shard // P  # 64
    n_groups = n_tiles // 4  # 16 vote groups of 4 tiles

    nc = bacc.Bacc("TRN2", target_bir_lowering=False)
    # x halves in [d, b] layout (prepared host-side during sharding)
    xht = nc.dram_tensor("xht", [D, b_shard], F16, kind="ExternalInput")
    xlt = nc.dram_tensor("xlt", [D, b_shard], F16, kind="ExternalInput")
    whl = nc.dram_tensor("whl", [D, 2 * MC], F16, kind="ExternalInput")
    bc4 = nc.dram_tensor("bc4", [2, 4 * MC], F16, kind="ExternalInput")
    # votes in SBUF staging layout [p, tile, c]; host de-interleaves
    y = nc.dram_tensor("y", [P, n_tiles * C], F16, kind="ExternalOutput")

    xht_r = xht.rearrange("(k p) b -> p k b", p=P)
    xlt_r = xlt.rearrange("(k p) b -> p k b", p=P)

    with tile.TileContext(nc) as tc:
        with (
            tc.tile_pool(name="consts", bufs=1) as consts,
            tc.tile_pool(name="xt", bufs=len(CHUNKS)) as xt_pool,
            tc.tile_pool(name="warm", bufs=1, space="PSUM") as warm_pool,
            tc.tile_pool(name="lg", bufs=3, space="PSUM") as lg_pool,
            tc.tile_pool(name="mx", bufs=3) as mx_pool,
            tc.tile_pool(name="eq", bufs=3) as eq_pool,
            tc.tile_pool(name="tsum", bufs=3) as tsum_pool,
            tc.tile_pool(name="stg", bufs=1) as stg_pool,
        ):
            # --- consts (tiny transfers, issued first on both queues) ---
            # whl_sb[:, k, 0, :] = wh k-half, [:, k, 1, :] = wl k-half; the
            # (h, c) pair is contiguous so rhs [p, 2, MC] fuses both passes
            whl_sb = consts.tile([P, 2, 2, MC], F16)
            nc.sync.dma_start(
                whl_sb, whl.rearrange("(k p) (h c) -> p k h c", p=P, h=2)
            )
            bc4_sb = consts.tile([2, 4 * MC], F16)
            nc.scalar.dma_start(bc4_sb, bc4[:])

            # --- all input DMAs issued up front ---
            # xt layout per chunk: [p, half(xh|xl), k, b]
            xts = []
            base = 0
            for ci, L in enumerate(CHUNKS):
                xt = xt_pool.tile([P, 2, 2, max(CHUNKS)], F16, name="xt")
                xts.append(xt)
                if ci == 0:
                    # k-split so the PE can start after the k=0 halves land
                    for k in range(2):
                        nc.sync.dma_start(
                            xt[:, 0, k, :L], xht_r[:, k, base : base + L]
                        )
                    for k in range(2):
                        nc.scalar.dma_start(
                            xt[:, 1, k, :L], xlt_r[:, k, base : base + L]
                        )
                else:
                    nc.sync.dma_start(
                        xt[:, 0, :, :L], xht_r[:, :, base : base + L]
                    )
                    nc.scalar.dma_start(
                        xt[:, 1, :, :L], xlt_r[:, :, base : base + L]
                    )
                base += L

            ones_w = consts.tile([2, 512], F16)
            nc.vector.memset(ones_w, 1.0)

            # --- PE warm-up while the first chunk is in flight ---
            warm = warm_pool.tile([P, 512], F32)
            for _ in range(WARMUP_MMS):
                nc.tensor.matmul(
                    warm, lhsT=ones_w[:, :P], rhs=ones_w, start=True, stop=True
                )

            stg = stg_pool.tile([P, n_tiles * C], F16)

            # global tile T -> (chunk index, within-chunk column)
            tile_map = []
            base = 0
            for ci, L in enumerate(CHUNKS):
                for t in range(L // P):
                    tile_map.append((ci, t * P))
                base += L

            # --- main pipeline: super-batches of 8 tiles (may span chunks) ---
            n_sb = n_tiles // 8
            for SB in range(n_sb):
                # lg: two bank-aligned 4-tile PSUM groups at [:, g, 0:320]
                lg = lg_pool.tile([P, 2, 512], F32, name="lg")
                for g in range(2):
                    # seed the accumulation group with the bias: every row
                    # of ones.T @ (bh4|bl4) is bh4+bl4
                    nc.tensor.matmul(
                        lg[:, g, : 4 * MC], lhsT=ones_w[:, :P], rhs=bc4_sb,
                        start=True, stop=False,
                    )
                # k-phased so phase 0 only needs the k=0 x halves
                for k in range(2):
                    for j in range(8):
                        g, o = j // 4, (j % 4) * MC
                        ci, col = tile_map[SB * 8 + j]
                        xt = xts[ci]
                        xh_c = xt[:, 0, k, col : col + P]
                        xl_c = xt[:, 1, k, col : col + P]
                        out = lg[:, g, o : o + MC]
                        last = k == 1 and (j % 4) == 3
                        # xh@wh + xh@wl in ONE matmul: the out AP's
                        # stride-0 h dim folds columns 80..159 onto
                        # 0..79, accumulating both products (start=False
                        # means every column-write accumulates)
                        nc.tensor.matmul(
                            out[:, None, :].broadcast_to([P, 2, MC]),
                            lhsT=xh_c, rhs=whl_sb[:, k],
                            start=False, stop=False,
                        )
                        nc.tensor.matmul(
                            out, lhsT=xl_c, rhs=whl_sb[:, k, 0, :],
                            start=False, stop=last,
                        )

                # votes: one DVE op-chain over both banks, except the final
                # super-batch where per-bank chains shorten the tail latency
                for gs, ge in ((0, 2),) if SB < n_sb - 1 else ((0, 1), (1, 2)):
                    ng = ge - gs
                    lgv = lg[:, gs:ge, : 4 * MC].rearrange(
                        "p g (t m c) -> p g t m c", m=M, c=C
                    )
                    mx = mx_pool.tile([P, 2, 4, M], F32, name="mx")
                    mxv = mx[:, gs:ge]
                    nc.vector.reduce_max(mxv, lgv, axis=mybir.AxisListType.X)
                    # mask in fp16, unit-stride (g, t, m, c) write
                    eq = eq_pool.tile([P, 2, 4, M, C], F16, name="eq")
                    eqv = eq[:, gs:ge]
                    nc.vector.tensor_tensor(
                        out=eqv,
                        in0=lgv,
                        in1=mxv[:, :, :, :, None].broadcast_to(
                            [P, ng, 4, M, C]
                        ),
                        op=mybir.AluOpType.is_ge,
                    )
                    # member-sum as a unit-stride fp16 add tree (2x DVE
                    # throughput vs a strided reduce); sums of <=8 ones are
                    # exact in fp16
                    t4 = tsum_pool.tile([P, 2, 4, 4, C], F16, name="t4")
                    nc.vector.tensor_tensor(
                        out=t4[:, gs:ge],
                        in0=eqv[:, :, :, 0:4, :], in1=eqv[:, :, :, 4:8, :],
                        op=mybir.AluOpType.add,
                    )
                    t2 = tsum_pool.tile([P, 2, 4, 2, C], F16, name="t2")
                    nc.vector.tensor_tensor(
                        out=t2[:, gs:ge],
                        in0=t4[:, gs:ge, :, 0:2, :], in1=t4[:, gs:ge, :, 2:4, :],
                        op=mybir.AluOpType.add,
                    )
                    nc.vector.tensor_tensor(
                        out=stg[:, SB * 8 * C + gs * 4 * C :][
                            :, : ng * 4 * C
                        ].rearrange("p (g t c) -> p g t c", g=ng, c=C),
                        in0=t2[:, gs:ge, :, 0, :],
                        in1=t2[:, gs:ge, :, 1, :],
                        op=mybir.AluOpType.add,
                    )

            nc.sync.dma_start(y[:], stg[:])
    nc.compile()
    return nc


_NC_CACHE: dict[int, bass.Bass] = {}


def _get_nc(b_shard: int) -> bass.Bass:
    if b_shard not in _NC_CACHE:
        _NC_CACHE[b_shard] = build_nc(b_shard)
    return _NC_CACHE[b_shard]


def _prep_inputs(x: np.ndarray, W: np.ndarray, b: np.ndarray):
    xf = np.asarray(x, dtype=np.float32)
    xh = xf.astype(np.float16)
    xl = (xf - xh.astype(np.float32)).astype(np.float16)
    parts = {
        "xht": np.ascontiguousarray(xh.T),
        "xlt": np.ascontiguousarray(xl.T),
    }
    # m-major columns: col index = 10*m + c; wh|wl concatenated per row
    wf = np.asarray(W, dtype=np.float32).transpose(1, 0, 2).reshape(D, MC)
    whf = wf.astype(np.float16)
    wlf = (wf - whf.astype(np.float32)).astype(np.float16)
    whlf = np.concatenate([whf, wlf], axis=1)
    bf = np.asarray(b, dtype=np.float32).reshape(MC)
    bh = bf.astype(np.float16)
    bl = (bf - bh.astype(np.float32)).astype(np.float16)
    bc4 = np.ascontiguousarray(
        np.stack([np.tile(bh, 4), np.tile(bl, 4)], axis=0)
    ).astype(np.float16)
    return parts, np.ascontiguousarray(whlf), bc4


def _postprocess(y_raw: np.ndarray) -> np.ndarray:
    # [p, tile*10] fp16 -> [tile*128, 10] f32 (values are small ints: exact)
    n_tiles = y_raw.shape[1] // C
    return (
        y_raw.reshape(P, n_tiles, C)
        .transpose(1, 0, 2)
        .reshape(n_tiles * P, C)
        .astype(np.float32)
    )


def kernel(x: np.ndarray, W: np.ndarray, b: np.ndarray, **_) -> np.ndarray:
    from concourse.bass_utils import run_bass_kernel_spmd

    assert x.shape == (B_FULL, D), x.shape
    parts, whlf, bc4 = _prep_inputs(x, W, b)

    nc = _get_nc(B_SHARD)
    in_maps = [
        {
            **{
                k: np.ascontiguousarray(v[:, i * B_SHARD : (i + 1) * B_SHARD])
                for k, v in parts.items()
            },
            "whl": whlf,
            "bc4": bc4,
        }
        for i in range(N_CORES)
    ]
    res = run_bass_kernel_spmd(nc, in_maps, core_ids=list(range(N_CORES)))
    return np.concatenate(
        [_postprocess(res.results[i]["y"]) for i in range(N_CORES)], axis=0
    )
